# revision 11
# baseline (speedup 1.0000x reference)
"""Trainium2 Bass kernel for nn_EnergyConditionedAtomConvolution.

Self-contained: takes FULL inputs (as in reference.setup_inputs()), shards
data-parallel over B across 8 NeuronCores, runs a Bass/Tile kernel per core,
gathers the full (64, 1024, 256) fp32 output.

Key structure (per core, 8 samples):
  Stage A: sparse per-edge msg + gate MLPs in feature-major layout
           (features on partitions, edge columns on free dim), edges padded
           to E_S columns per sample; segment-sum -> m_abs^T [256, 8].
  Stage B: e_gate MLP on e_feat -> e_gate^T [256, 1024] (replicated).
  Stage C: per sample: x^T = e_gate^T * m_abs (per-partition scale),
           then 256->512->256 MLP; out^T [256, 1024] -> DRAM.
"""

import numpy as np

import concourse.bacc as bacc
import concourse.mybir as mybir
from concourse.tile import TileContext
from concourse.bass_utils import run_bass_kernel_spmd

# Problem dims (hardcoded; kernel.py may not read spec/reference).
B, N, ATOM_DIM = 64, 512, 256
E_DIM, HIDDEN, LATENT = 32, 512, 256
RBF_DIM, Z_EMB_DIM = 16, 32
N_E, E_ATT = 1024, 4096
CUTOFF = 6.0
N_CORES = 8
BPC = B // N_CORES  # samples per core

MSG_IN = ATOM_DIM + Z_EMB_DIM + 1 + RBF_DIM  # 305
GATE_IN_DEV = 2 * ATOM_DIM + Z_EMB_DIM + 1 + RBF_DIM  # 561 (zr rows zeroed)

F32 = mybir.dt.float32
F32R = mybir.dt.float32r
BF16 = mybir.dt.bfloat16

_PROG_CACHE: dict = {}
LAST_RESULT = None


def _ftiles(total, step=512):
    return [(i, min(i + step, total)) for i in range(0, total, step)]


def _kchunks(total, step=128):
    return [(i, min(i + step, total)) for i in range(0, total, step)]


def _build_program(e_s: int, dt_mm: str):
    """Build + compile the SPMD single-core program. dt_mm: 'f32r'|'f32'|'bf16'."""
    ne = BPC * e_s
    mmdt = {"bf16": BF16, "f32r": F32R, "f32": F32}[dt_mm]

    nc = bacc.Bacc("TRN2", target_bir_lowering=False, debug=False,
                   num_devices=N_CORES)

    def dram(name, shape, dtype=None, out=False):
        return nc.dram_tensor(
            name, list(shape), dtype or mmdt,
            kind="ExternalOutput" if out else "ExternalInput")

    # ---- DRAM I/O ----
    hT_d = dram("hT", [ATOM_DIM, ne])
    zrT_d = dram("zrT", [Z_EMB_DIM, ne])
    isabs_d = dram("isabs", [1, ne])
    d16_d = dram("d16", [RBF_DIM, ne], F32)
    d1_d = dram("d1", [1, ne], F32)
    valid_d = dram("valid", [1, ne], F32)
    habsT_d = dram("habsT", [ATOM_DIM, BPC], F32)
    efT_d = dram("efT", [E_DIM, N_E])
    negoff_d = dram("negoff", [RBF_DIM, 1], F32)

    mw1_d = dram("mw1", [MSG_IN, HIDDEN])
    mw2_d = dram("mw2", [HIDDEN, HIDDEN])
    mw3_d = dram("mw3", [HIDDEN, LATENT])
    mb1_d = dram("mb1", [128, 4], F32)
    mb2_d = dram("mb2", [128, 4], F32)
    mb3_d = dram("mb3", [128, 2], F32)
    gw1_d = dram("gw1", [GATE_IN_DEV, HIDDEN])
    gw2_d = dram("gw2", [HIDDEN, 1])
    gb1_d = dram("gb1", [128, 4], F32)
    gb2_d = dram("gb2", [1, 1], F32)
    ew1_d = dram("ew1", [E_DIM, HIDDEN])
    ew2_d = dram("ew2", [HIDDEN, HIDDEN])
    ew3_d = dram("ew3", [HIDDEN, LATENT])
    eb1_d = dram("eb1", [128, 4], F32)
    eb2_d = dram("eb2", [128, 4], F32)
    eb3_d = dram("eb3", [128, 2], F32)
    ow1_d = dram("ow1", [LATENT, HIDDEN])
    ow2_d = dram("ow2", [HIDDEN, LATENT])
    ob1_d = dram("ob1", [128, 4], F32)
    ob2_d = dram("ob2", [128, 2], F32)

    outT_d = dram("outT", [BPC, LATENT, N_E], F32, out=True)

    AF = mybir.ActivationFunctionType
    PI = float(np.pi)
    offs = np.linspace(0.0, CUTOFF, RBF_DIM)
    rbf_coeff = float(-0.5 / (offs[1] - offs[0]) ** 2)

    def mm(ps, lhsT, rhs, start, stop):
        nc.tensor.matmul(ps, lhsT, rhs, start=start, stop=stop)

    with TileContext(nc) as tc:
        with (
            tc.tile_pool(name="w", bufs=1) as wp,
            tc.tile_pool(name="a", bufs=1) as ap,
            tc.tile_pool(name="ps", bufs=8, space="PSUM") as psp,
            tc.tile_pool(name="c", bufs=1) as cp,
            tc.tile_pool(name="rot", bufs=2) as rot,
        ):
            def T(pool, shape, dtype, tag):
                return pool.tile(shape, dtype, tag=tag, name=tag)

            def load_w(d, tag):
                tiles = []
                for i, (k0, k1) in enumerate(_kchunks(d.shape[0])):
                    t = wp.tile([k1 - k0, d.shape[1]], mmdt, tag=f"{tag}{i}")
                    nc.sync.dma_start(out=t[:, :], in_=d[k0:k1, :])
                    tiles.append(t)
                return tiles

            def load_b(d, tag):
                t = wp.tile(list(d.shape), F32, tag=tag)
                nc.sync.dma_start(out=t[:, :], in_=d[:, :])
                return t

            mw1 = load_w(mw1_d, "mw1")
            mw2 = load_w(mw2_d, "mw2")
            mw3 = load_w(mw3_d, "mw3")
            gw1 = load_w(gw1_d, "gw1")
            gw2 = load_w(gw2_d, "gw2")
            ew1 = load_w(ew1_d, "ew1")
            ew2 = load_w(ew2_d, "ew2")
            ew3 = load_w(ew3_d, "ew3")
            ow1 = load_w(ow1_d, "ow1")
            ow2 = load_w(ow2_d, "ow2")
            mb1 = load_b(mb1_d, "mb1")
            mb2 = load_b(mb2_d, "mb2")
            mb3 = load_b(mb3_d, "mb3")
            gb1 = load_b(gb1_d, "gb1")
            gb2 = load_b(gb2_d, "gb2")
            eb1 = load_b(eb1_d, "eb1")
            eb2 = load_b(eb2_d, "eb2")
            eb3 = load_b(eb3_d, "eb3")
            ob1 = load_b(ob1_d, "ob1")
            ob2 = load_b(ob2_d, "ob2")
            negoff = load_b(negoff_d, "negoff")

            # ---- Stage A: edge features ----
            fa = T(ap, [128, ne], mmdt, "fa")
            fb = T(ap, [128, ne], mmdt, "fb")
            fc = T(ap, [49, ne], mmdt, "fc")
            nc.sync.dma_start(out=fa[:, :], in_=hT_d[0:128, :])
            nc.sync.dma_start(out=fb[:, :], in_=hT_d[128:256, :])
            nc.sync.dma_start(out=fc[16:48, :], in_=zrT_d[:, :])
            nc.sync.dma_start(out=fc[48:49, :], in_=isabs_d[:, :])

            d16 = T(ap, [RBF_DIM, ne], F32, "d16")
            nc.sync.dma_start(out=d16[:, :], in_=d16_d[:, :])
            nc.scalar.activation(d16[:, :], d16[:, :], AF.Square,
                                 bias=negoff[:, 0:1], scale=1.0)
            nc.scalar.activation(fc[0:16, :], d16[:, :], AF.Exp,
                                 bias=0.0, scale=rbf_coeff)

            # h_abs broadcast blocks (gate input chunks 1-2)
            habs = []
            for i in range(2):
                t = T(ap, [128, BPC], F32, f"habs{i}")
                nc.sync.dma_start(out=t[:, :], in_=habsT_d[i * 128:(i + 1) * 128, :])
                habs.append(t)
            ones = T(ap, [128, e_s], mmdt, "ones")
            nc.vector.memset(ones[:, :], 1.0)
            ga = T(ap, [128, ne], mmdt, "ga")
            gb = T(ap, [128, ne], mmdt, "gb")
            for s in range(BPC):
                c0, c1 = s * e_s, (s + 1) * e_s
                nc.vector.tensor_scalar_mul(ga[:, c0:c1], ones[:, :], habs[0][:, s:s + 1])
                nc.vector.tensor_scalar_mul(gb[:, c0:c1], ones[:, :], habs[1][:, s:s + 1])

            NT = _ftiles(ne)

            def layer(k_srcs, w_tiles, n_out, out_tag, act, bias_t, out_dt,
                      nts=NT, kparts=None):
                """k_srcs: list of (tile, psize). Returns list of out chunk tiles."""
                mchunks = _kchunks(n_out)
                outs = [T(ap, [m1 - m0, nts[-1][1]], out_dt, f"{out_tag}{mi}")
                        for mi, (m0, m1) in enumerate(mchunks)]
                for (n0, n1) in nts:
                    for mi, (m0, m1) in enumerate(mchunks):
                        ps = T(psp, [m1 - m0, n1 - n0], F32, "ps")
                        for ki, (src, kp) in enumerate(k_srcs):
                            mm(ps[:, :], w_tiles[ki][:, m0:m1], src[0:kp, n0:n1],
                               start=(ki == 0), stop=(ki == len(k_srcs) - 1))
                        nc.scalar.activation(outs[mi][:, n0:n1], ps[:, :], act,
                                             bias=bias_t[0:m1 - m0, mi:mi + 1],
                                             scale=1.0)
                return outs

            # msg MLP: 305 -> 512 -> 512 -> 256
            msg_src = [(fa, 128), (fb, 128), (fc, 49)]
            mh1 = layer(msg_src, mw1, HIDDEN, "mh1", AF.Silu, mb1, mmdt)
            mh2 = layer([(t, 128) for t in mh1], mw2, HIDDEN, "mh2", AF.Silu, mb2, mmdt)

            # gate MLP: 561 -> 512 -> 1 (rows: habs, h, zr(zeroed), isabs, rbf)
            gate_src = [(ga, 128), (gb, 128), (fa, 128), (fb, 128), (fc, 49)]
            gh1 = layer(gate_src, gw1, HIDDEN, "gh1", AF.Silu, gb1, mmdt)
            glog = T(cp, [1, ne], F32, "glog")
            for (n0, n1) in NT:
                ps = T(psp, [1, n1 - n0], F32, "ps")
                for ki in range(4):
                    mm(ps[:, :], gw2[ki][:, 0:1], gh1[ki][:, n0:n1],
                       start=(ki == 0), stop=(ki == 3))
                nc.scalar.activation(glog[:, n0:n1], ps[:, :], AF.Sigmoid,
                                     bias=gb2[0:1, 0:1], scale=1.0)

            # env and per-edge weight w = env * gate * valid
            d1 = T(cp, [1, ne], F32, "d1")
            valid = T(cp, [1, ne], F32, "valid")
            nc.sync.dma_start(out=d1[:, :], in_=d1_d[:, :])
            nc.sync.dma_start(out=valid[:, :], in_=valid_d[:, :])
            cosd = T(cp, [1, ne], F32, "cosd")
            env = T(cp, [1, ne], F32, "env")
            halfpi = T(cp, [1, 1], F32, "halfpi")
            nc.vector.memset(halfpi[:, :], PI / 2.0)
            nc.scalar.activation(cosd[:, :], d1[:, :], AF.Sin,
                                 bias=halfpi[0:1, 0:1], scale=PI / CUTOFF)
            nc.scalar.activation(env[:, :], cosd[:, :], AF.Copy,
                                 bias=0.5, scale=0.5)
            wrow = T(cp, [1, ne], F32, "wrow")
            nc.vector.tensor_mul(wrow[:, :], env[:, :], glog[:, :])
            nc.vector.tensor_mul(wrow[:, :], wrow[:, :], valid[:, :])
            wrow_mm = wrow
            if dt_mm != "f32":
                wrow_mm = T(cp, [1, ne], mmdt, "wrowb")
                nc.vector.tensor_copy(wrow_mm[:, :], wrow[:, :])

            # broadcast w to 128 partitions via K=1 matmul with ones
            ones1 = T(cp, [1, 128], mmdt, "ones1")
            nc.vector.memset(ones1[:, :], 1.0)
            w128 = T(cp, [128, ne], F32, "w128")
            for (n0, n1) in NT:
                ps = T(psp, [128, n1 - n0], F32, "ps")
                mm(ps[:, :], ones1[:, :], wrow_mm[:, n0:n1], start=True, stop=True)
                nc.vector.tensor_copy(w128[:, n0:n1], ps[:, :])

            # msg L3 (512 -> 256), weight+mask, segment reduce -> m_abs^T
            mabsT = []
            for mi, (m0, m1) in enumerate(_kchunks(LATENT)):
                msgw = T(ap, [128, ne], F32, f"msgw{mi}")
                for (n0, n1) in NT:
                    ps = T(psp, [128, n1 - n0], F32, "ps")
                    for ki in range(4):
                        mm(ps[:, :], mw3[ki][:, m0:m1], mh2[ki][:, n0:n1],
                           start=(ki == 0), stop=(ki == 3))
                    nc.scalar.activation(ps[:, :], ps[:, :], AF.Identity,
                                         bias=mb3[0:128, mi:mi + 1], scale=1.0)
                    nc.vector.tensor_mul(msgw[:, n0:n1], ps[:, :], w128[:, n0:n1])
                mt = T(cp, [128, BPC], F32, f"mabsT{mi}")
                nc.vector.tensor_reduce(
                    mt[:, :], msgw[:, :].rearrange("p (s e) -> p s e", e=e_s),
                    axis=mybir.AxisListType.X, op=mybir.AluOpType.add)
                mabsT.append(mt)

            # ---- Stage B: e_gate MLP 32 -> 512 -> 512 -> 256 on e_feat ----
            ef = T(ap, [E_DIM, N_E], mmdt, "ef")
            nc.sync.dma_start(out=ef[:, :], in_=efT_d[:, :])
            NTE = _ftiles(N_E)
            eh1 = layer([(ef, E_DIM)], ew1, HIDDEN, "mh1", AF.Silu, eb1, mmdt, nts=NTE)
            eh2 = layer([(t, 128) for t in eh1], ew2, HIDDEN, "mh2", AF.Silu, eb2,
                        mmdt, nts=NTE)
            egT = layer([(t, 128) for t in eh2], ew3, LATENT, "egT", AF.Identity,
                        eb3, mmdt, nts=NTE)

            # ---- Stage C: per sample out MLP ----
            for s in range(BPC):
                xT = [T(rot, [128, N_E], mmdt, f"xT{i}") for i in range(2)]
                for i in range(2):
                    nc.scalar.activation(xT[i][:, :], egT[i][:, :], AF.Copy,
                                         bias=0.0, scale=mabsT[i][:, s:s + 1])
                for (n0, n1) in NTE:
                    h1c = [T(rot, [128, n1 - n0], mmdt, f"h1c{mi}")
                           for mi in range(4)]
                    for mi in range(4):
                        ps = T(psp, [128, n1 - n0], F32, "ps")
                        for ki in range(2):
                            mm(ps[:, :], ow1[ki][:, mi * 128:(mi + 1) * 128],
                               xT[ki][:, n0:n1], start=(ki == 0), stop=(ki == 1))
                        nc.scalar.activation(h1c[mi][:, :], ps[:, :], AF.Silu,
                                             bias=ob1[0:128, mi:mi + 1], scale=1.0)
                    for m2 in range(2):
                        ps = T(psp, [128, n1 - n0], F32, "ps")
                        for ki in range(4):
                            mm(ps[:, :], ow2[ki][:, m2 * 128:(m2 + 1) * 128],
                               h1c[ki][:, :], start=(ki == 0), stop=(ki == 3))
                        osb = T(rot, [128, n1 - n0], F32, "osb")
                        nc.vector.tensor_scalar_add(osb[:, :], ps[:, :],
                                                    ob2[0:128, m2:m2 + 1])
                        nc.sync.dma_start(
                            out=outT_d[s, m2 * 128:(m2 + 1) * 128, n0:n1],
                            in_=osb[:, :])

    nc.compile()
    return nc


def _np32(x):
    return np.asarray(x, dtype=np.float32)


def _cast_mm(x, dt_mm):
    if dt_mm == "bf16":
        import ml_dtypes
        return np.asarray(x, dtype=ml_dtypes.bfloat16)
    return np.ascontiguousarray(_np32(x))


def _b_reshape(b, nchunks):
    b = _np32(b).reshape(-1)
    pad = nchunks * 128 - b.size
    if pad:
        b = np.concatenate([b, np.zeros(pad, np.float32)])
    return np.ascontiguousarray(b.reshape(nchunks, 128).T)


def kernel(dt_mm: str = "f32r", **inputs) -> np.ndarray:
    h = _np32(inputs["h"])
    z = np.asarray(inputs["z"])
    mask = np.asarray(inputs["mask"])
    e_feat = _np32(inputs["e_feat"])
    absorber_index = np.asarray(inputs["absorber_index"])
    att_dst = np.asarray(inputs["att_dst"])
    att_dist = _np32(inputs["att_dist"])
    z_emb = _np32(inputs["z_emb"])
    msg_params = [( _np32(w), _np32(b)) for w, b in inputs["msg_params"]]
    gate_params = [(_np32(w), _np32(b)) for w, b in inputs["gate_params"]]
    egate_params = [(_np32(w), _np32(b)) for w, b in inputs["egate_params"]]
    out_params = [(_np32(w), _np32(b)) for w, b in inputs["out_params"]]

    # --- edge routing (host): sample id, within-sample rank ---
    bvec = att_dst // N
    nvec = att_dst % N
    counts = np.bincount(bvec, minlength=B)
    e_s = max(96, int(np.ceil(counts.max() / 32.0)) * 32)
    ne = BPC * e_s

    key = (e_s, dt_mm)
    if key not in _PROG_CACHE:
        _PROG_CACHE[key] = _build_program(e_s, dt_mm)
    nc = _PROG_CACHE[key]

    order = np.argsort(bvec, kind="stable")
    sb = bvec[order]
    sn = nvec[order]
    sdist = att_dist[order]
    starts = np.searchsorted(sb, np.arange(B))
    rank = np.arange(E_ATT) - starts[sb]
    col = (sb % BPC) * e_s + rank  # column within the core's edge block
    core = sb // BPC

    valid_e = mask[sb, sn].astype(np.float32)
    isabs_e = (sn == absorber_index[sb]).astype(np.float32)
    zr_e = z_emb[z[sb, sn]]  # (E_ATT, 32)
    h_e = h[sb, sn]  # (E_ATT, 256)
    habs = h[np.arange(B), absorber_index]  # (B, 256)

    # --- shared (replicated) weight arrays ---
    offs = np.linspace(0.0, CUTOFF, RBF_DIM).astype(np.float32)
    gW = gate_params[0][0]
    gw1 = np.zeros((GATE_IN_DEV, HIDDEN), np.float32)
    gw1[0:512] = gW[0:512]
    gw1[512:528] = gW[512:528]   # rbf rows
    gw1[560] = gW[528]           # isabs row


    shared = {
        "efT": _cast_mm(e_feat.T, dt_mm),
        "negoff": np.ascontiguousarray(-offs.reshape(RBF_DIM, 1)),
        "mw1": _cast_mm(np.concatenate([
            msg_params[0][0][0:256],
            msg_params[0][0][289:305],   # rbf rows
            msg_params[0][0][256:288],   # zr rows
            msg_params[0][0][288:289],   # isabs row
        ]), dt_mm),
        "mw2": _cast_mm(msg_params[1][0], dt_mm),
        "mw3": _cast_mm(msg_params[2][0], dt_mm),
        "mb1": _b_reshape(msg_params[0][1], 4),
        "mb2": _b_reshape(msg_params[1][1], 4),
        "mb3": _b_reshape(msg_params[2][1], 2),
        "gw1": _cast_mm(gw1, dt_mm),
        "gw2": _cast_mm(gate_params[1][0], dt_mm),
        "gb1": _b_reshape(gate_params[0][1], 4),
        "gb2": _np32(gate_params[1][1]).reshape(1, 1),
        "ew1": _cast_mm(egate_params[0][0], dt_mm),
        "ew2": _cast_mm(egate_params[1][0], dt_mm),
        "ew3": _cast_mm(egate_params[2][0], dt_mm),
        "eb1": _b_reshape(egate_params[0][1], 4),
        "eb2": _b_reshape(egate_params[1][1], 4),
        "eb3": _b_reshape(egate_params[2][1], 2),
        "ow1": _cast_mm(out_params[0][0], dt_mm),
        "ow2": _cast_mm(out_params[1][0], dt_mm),
        "ob1": _b_reshape(out_params[0][1], 4),
        "ob2": _b_reshape(out_params[1][1], 2),
    }

    in_maps = []
    for c in range(N_CORES):
        sel = core == c
        cc = col[sel]
        hT = np.zeros((ne, ATOM_DIM), np.float32)
        zrT = np.zeros((ne, Z_EMB_DIM), np.float32)
        isabs = np.zeros((1, ne), np.float32)
        d1 = np.zeros((1, ne), np.float32)
        valid = np.zeros((1, ne), np.float32)
        hT[cc] = h_e[sel]
        zrT[cc] = zr_e[sel]
        isabs[0, cc] = isabs_e[sel]
        d1[0, cc] = sdist[sel]
        valid[0, cc] = valid_e[sel]
        m = {
            "hT": _cast_mm(hT.T, dt_mm),
            "zrT": _cast_mm(zrT.T, dt_mm),
            "isabs": _cast_mm(isabs, dt_mm),
            "d16": np.ascontiguousarray(np.repeat(d1, RBF_DIM, axis=0)),
            "d1": d1,
            "valid": valid,
            "habsT": np.ascontiguousarray(habs[c * BPC:(c + 1) * BPC].T),
        }
        m.update(shared)
        in_maps.append(m)

    res = run_bass_kernel_spmd(nc, in_maps, list(range(N_CORES)))
    global LAST_RESULT
    LAST_RESULT = res
    out = np.empty((B, N_E, LATENT), np.float32)
    for c in range(N_CORES):
        o = res.results[c]["outT"]  # [BPC, 256, 1024]
        out[c * BPC:(c + 1) * BPC] = np.transpose(o, (0, 2, 1))
    return out
